# revision 1
# baseline (speedup 1.0000x reference)
"""Trainium2 Bass kernel for nn_CausalSelfAttention_73358041415963.

Math (literal reference semantics):
  Q/K/V = per-head projections of X;  S = Q @ K^T (no scale, no mask)
  A = softmax(S, axis=QUERY)  -> each key-column normalized over queries
  AV = A @ V;  literal reshape (B,H,N,DV)->(B,N,H*DV);  out = AV_r @ W_O

Key structural facts exploited:
  * softmax over the query axis i means A = E / colsum(E) with E = exp(S);
    AV = E @ (V / n[:,None]) where n[jk] = sum_i E[i, jk] -- normalization
    folds into V rows, no pass over the big E matrix.
  * the literal reshape maps head h to output rows n' in [h*128,(h+1)*128),
    so head-sharding needs NO collectives: each core owns 2 heads = 256
    output rows per batch.

Sharding: 8 cores x 2 heads. Each core gets full X, its 2 heads' W_Q/W_K/W_V
(packed [D,128]), full W_O. Core c returns output rows [256c, 256c+256).
"""

import numpy as np

import concourse.tile as tile
from concourse import bacc, mybir
from concourse.bass_utils import run_bass_kernel_spmd
from concourse.masks import make_identity

F32 = mybir.dt.float32
F32R = mybir.dt.float32r
P = 128
AF = mybir.ActivationFunctionType

# float32r = single-pass fp32 matmul (4x faster than plain fp32 on the PE,
# slightly reduced internal precision). Flags allow selective fallback.
F32R_PROJ = True
F32R_S = True
F32R_AV = True
F32R_WO = True


DT_PROJ = F32R if F32R_PROJ else F32
DT_S = F32R if F32R_S else F32
DT_AV = F32R if F32R_AV else F32
DT_WO = F32R if F32R_WO else F32


def build_attn(tc, X, WQ, WK, WV, WO, O, N, D, DOUT):
    """Emit the per-core kernel into TileContext tc.

    X:  [2, N, D]    (full input, fp32)
    WQ/WK/WV: [D, 128]   2 local heads packed along the last axis
    WO: [16*64, DOUT]
    O:  [2, 2*(N//16), DOUT]   output rows for the 2 local heads
    """
    nc = tc.nc
    B, HL, SG = 2, 2, 16
    DCH = D // 128        # contraction chunks over model dim
    NCH = N // 512        # 512-wide chunks of sequence
    JKB = N // 128        # key blocks
    IHALF = N // 2        # scores processed in two i-halves
    CS = min(512, IHALF)  # matmul free-dim chunk
    NCPH = IHALF // CS    # chunks per half
    # PSUM accumulation groups must own whole 2KB banks (start=True lazily
    # zeroes the full bank for the written partitions), so chunks must be
    # 512 fp32 elements.
    assert CS == 512, "need N % 1024 == 0 so avp chunks are bank-aligned"
    R = N // 16           # output rows per head

    with (
        tc.tile_pool(name="persist", bufs=1) as pp,
    ):
        ident = pp.tile([P, P], F32, tag="ident", name="ident")
        make_identity(nc, ident)
        if F32R_PROJ:
            identr = pp.tile([P, P], DT_PROJ, tag="identr", name="identr")
            nc.vector.tensor_copy(identr, ident)
        else:
            identr = ident
        # Dummy exp: forces the ACT Exp table load during the prologue
        # instead of at the first real score-exp.
        warm = pp.tile([P, 1], F32, tag="warm", name="warm")
        nc.scalar.activation(warm, ident[:, 0:1], AF.Exp)
        wq_sb = pp.tile([P, DCH, P], DT_PROJ, tag="wq", name="wq_sb")
        wk_sb = pp.tile([P, DCH, P], DT_PROJ, tag="wk", name="wk_sb")
        wv_sb = pp.tile([P, DCH, P], DT_PROJ, tag="wv", name="wv_sb")
        nc.sync.dma_start(wq_sb, WQ.rearrange("(dc p) m -> p dc m", p=P))
        nc.sync.dma_start(wk_sb, WK.rearrange("(dc p) m -> p dc m", p=P))
        nc.sync.dma_start(wv_sb, WV.rearrange("(dc p) m -> p dc m", p=P))

        qT, kT, v_sb, avr = [], [], [], []
        for b in range(B):
            qT.append(pp.tile([P, N], DT_S, tag=f"qT{b}", name=f"qT{b}"))
            kT.append(pp.tile([P, N], DT_S, tag=f"kT{b}", name=f"kT{b}"))
            v_sb.append(pp.tile([P, JKB, P], F32, tag=f"v{b}", name=f"v{b}"))
            avr.append(pp.tile([P, N], DT_WO, tag=f"avr{b}", name=f"avr{b}"))

        # ---------------- Phase P: X^T, projections ----------------
        with (
            tc.tile_pool(name="pP", bufs=1) as sp,
            tc.tile_pool(name="pPps", bufs=1, space="PSUM") as pps,
        ):
            def emit_vtrans(pend):
                vt_pend, b_pend, nch_pend = pend
                tp2 = pps.tile([P, 512], F32, tag="tp", bufs=4, name="tp2")
                for ns in range(4):
                    nc.tensor.transpose(
                        tp2[:, ns * 128 : (ns + 1) * 128],
                        vt_pend[:, ns * 128 : (ns + 1) * 128],
                        ident,
                    )
                nc.vector.tensor_copy(
                    v_sb[b_pend][:, nch_pend * 4 : (nch_pend + 1) * 4, :], tp2
                )

            pending_vt = None
            for b in range(B):
                for nch in range(NCH):
                    xns = []
                    for ns in range(4):
                        xn = sp.tile([P, D], DT_PROJ, tag="xn", bufs=8, name="xn")
                        n0 = nch * 512 + ns * 128
                        nc.sync.dma_start(xn, X[b, n0 : n0 + 128, :])
                        xns.append(xn)
                    if pending_vt is not None:
                        emit_vtrans(pending_vt)
                        pending_vt = None
                    # Q^T, K^T, V^T for this n-chunk (2 heads packed on
                    # partitions). Per d-chunk: transpose X block, copy to
                    # SBUF, immediately run the 3 accumulating projections.
                    qt_ps = pps.tile([P, 512], F32, tag="qk", bufs=4, name="qt_ps")
                    kt_ps = pps.tile([P, 512], F32, tag="qk", bufs=4, name="kt_ps")
                    vt_ps = pps.tile([P, 512], F32, tag="qk", bufs=4, name="vt_ps")
                    # Software-pipelined by one d-chunk: the PE queue is
                    # in-order, so the projection matmuls for chunk dc are
                    # emitted after chunk dc+1's transposes — the PE streams
                    # transposes while the copy for dc drains on DVE/ACT.
                    def emit_mms(dc, xtc):
                        nc.tensor.matmul(
                            qt_ps, wq_sb[:, dc, :], xtc,
                            start=(dc == 0), stop=(dc == DCH - 1),
                        )
                        nc.tensor.matmul(
                            kt_ps, wk_sb[:, dc, :], xtc,
                            start=(dc == 0), stop=(dc == DCH - 1),
                        )
                        nc.tensor.matmul(
                            vt_ps, wv_sb[:, dc, :], xtc,
                            start=(dc == 0), stop=(dc == DCH - 1),
                        )

                    prev = None
                    for dc in range(DCH):
                        tp = pps.tile([P, 512], DT_PROJ, tag="tp", bufs=4, name="tp")
                        for ns in range(4):
                            nc.tensor.transpose(
                                tp[:, ns * 128 : (ns + 1) * 128],
                                xns[ns][:, dc * 128 : (dc + 1) * 128],
                                identr,
                            )
                        xtc = sp.tile([P, 512], DT_PROJ, tag="xtc", bufs=6, name="xtc")
                        if dc % 2 == 0:
                            nc.vector.tensor_copy(xtc, tp)
                        else:
                            nc.scalar.copy(xtc, tp)
                        if prev is not None:
                            emit_mms(dc - 1, prev)
                        prev = xtc
                    emit_mms(DCH - 1, prev)
                    nc.vector.tensor_copy(qT[b][:, nch * 512 : (nch + 1) * 512], qt_ps)
                    nc.vector.tensor_copy(kT[b][:, nch * 512 : (nch + 1) * 512], kt_ps)
                    # V^T -> V natural via PE transposes, deferred to the
                    # start of the next chunk so the vt_sb drain never
                    # stalls the in-order PE queue.
                    vt_sb = sp.tile([P, 512], F32, tag="vt", bufs=3, name="vt_sb")
                    nc.scalar.copy(vt_sb, vt_ps)
                    pending_vt = (vt_sb, b, nch)

            if pending_vt is not None:
                emit_vtrans(pending_vt)
                pending_vt = None

        # Pre-issue the replicated W_O loads (no upstream deps -> their DMAs
        # overlap the projection/attention phases; bufs=2 pipelines the rest).
        OC = 256  # output column chunk (f32r still full-rate at N=256)
        wo_src = WO.rearrange("(s dv) d -> dv s d", dv=64)
        wo_tiles = []
        for dh in range(DOUT // OC):
            wo_t = pp.tile([P, SG, OC], DT_WO, tag="wo", bufs=4, name="wo_t")
            nc.sync.dma_start(wo_t[0:64], wo_src[:, :, dh * OC : (dh + 1) * OC])
            nc.sync.dma_start(wo_t[64:128], wo_t[0:64])
            wo_tiles.append(wo_t)

        # ---------------- Phase A: scores, exp, AV ----------------
        with (
            tc.tile_pool(name="pA", bufs=1) as ab,
            tc.tile_pool(name="pAps", bufs=1, space="PSUM") as aps,
        ):
            def emit_wo_chunk(wb, idx):
                # Output projection chunk (one (dh, head) pair) for batch
                # wb; op tiles borrow "st"-tag PSUM slots (no
                # pool-transition barrier after attention).
                dh, h = idx // HL, idx % HL
                wo_t = wo_tiles[dh]
                avv = avr[wb].rearrange("p (r s) -> p s r", s=SG)
                hs = slice(h * 64, (h + 1) * 64)
                opf = aps.tile([P, IHALF], F32, tag="st", bufs=2, name="opf")
                op = opf[:R, :OC]
                for s in range(SG):
                    nc.tensor.matmul(
                        op,
                        avv[hs, s, :],
                        wo_t[hs, s, :],
                        start=(s == 0), stop=(s == SG - 1),
                    )
                o_t = pp.tile([R, OC], F32, tag="ot", bufs=2, name="o_t")
                nc.scalar.copy(o_t, op)
                nc.sync.dma_start(
                    O[wb, h * R : (h + 1) * R, dh * OC : (dh + 1) * OC], o_t
                )

            for b in range(B):
                avp = aps.tile([P, N], F32, tag="avp", bufs=1, name="avp")
                for jk in range(JKB):
                    es = {}
                    nsum = {}
                    for h in range(HL):
                        nsum[h] = ab.tile([P, 2], F32, tag="nsum", bufs=4, name="nsum")
                    for half in range(2):
                        st = {}
                        for h in range(HL):
                            st[h] = aps.tile(
                                [P, IHALF], F32, tag="st", bufs=2, name="st"
                            )
                        for c in range(NCPH):
                            for h in range(HL):
                                hs = slice(h * 64, (h + 1) * 64)
                                i0 = half * IHALF + c * CS
                                nc.tensor.matmul(
                                    st[h][:, c * CS : (c + 1) * CS],
                                    kT[b][hs, jk * 128 : (jk + 1) * 128],
                                    qT[b][hs, i0 : i0 + CS],
                                    start=True, stop=True,
                                )
                        for h in range(HL):
                            e = ab.tile([P, IHALF], DT_AV, tag="e", bufs=8, name="e")
                            nc.scalar.activation(
                                e, st[h], AF.Exp,
                                accum_out=nsum[h][:, half : half + 1],
                            )
                            es[(h, half)] = e
                    # Per-head lhsT padded with zeros on the other head's
                    # columns: AV matmuls then write all 128 partitions
                    # (f32r matmuls reject PSUM outputs at partition base 64,
                    # and the zero half just accumulates +0).
                    vsp = {}
                    for h in range(HL):
                        hs = slice(h * 64, (h + 1) * 64)
                        zs = slice((1 - h) * 64, (2 - h) * 64)
                        n1 = ab.tile([P, 1], F32, tag="n1", bufs=4, name="n1")
                        nc.vector.reduce_sum(n1, nsum[h], axis=mybir.AxisListType.X)
                        nr = ab.tile([P, 1], F32, tag="nr", bufs=4, name="nr")
                        nc.vector.reciprocal(nr, n1)
                        vs = ab.tile([P, P], DT_AV, tag="vs", bufs=6, name="vs")
                        nc.vector.tensor_scalar_mul(vs[:, zs], v_sb[b][:, jk, zs], 0.0)
                        nc.vector.tensor_scalar_mul(
                            vs[:, hs], v_sb[b][:, jk, hs], nr
                        )
                        vsp[h] = vs
                    for half in range(2):
                        for c in range(NCPH):
                            for h in range(HL):
                                i0 = half * IHALF + c * CS
                                nc.tensor.matmul(
                                    avp[:, i0 : i0 + CS],
                                    vsp[h],
                                    es[(h, half)][:, c * CS : (c + 1) * CS],
                                    start=(jk == 0 and h == 0),
                                    stop=(jk == JKB - 1 and h == HL - 1),
                                    skip_group_check=True,
                                )
                if b == B - 1:
                    # Split the last avr drain across DVE+ACT: it gates the
                    # W_O tail and ACT is idle by then.
                    nc.vector.tensor_copy(avr[b][:, : N // 2], avp[:, : N // 2])
                    nc.scalar.copy(avr[b][:, N // 2 :], avp[:, N // 2 :])
                else:
                    nc.vector.tensor_copy(avr[b], avp)
            for wb in range(B):
                for idx in range((DOUT // OC) * HL):
                    emit_wo_chunk(wb, idx)


def build_nc(N=2048, D=1024, DOUT=1024, enable_asserts=False):
    """Build and compile the per-core Bass module. Returns nc."""
    nc = bacc.Bacc(
        "TRN2",
        target_bir_lowering=False,
        debug=False,
        enable_asserts=enable_asserts,
    )
    R = N // 16
    X = nc.dram_tensor("X", [2, N, D], DT_PROJ, kind="ExternalInput").ap()
    WQ = nc.dram_tensor("WQ", [D, 128], DT_PROJ, kind="ExternalInput").ap()
    WK = nc.dram_tensor("WK", [D, 128], DT_PROJ, kind="ExternalInput").ap()
    WV = nc.dram_tensor("WV", [D, 128], DT_PROJ, kind="ExternalInput").ap()
    WO = nc.dram_tensor("WO", [16 * 64, DOUT], DT_WO, kind="ExternalInput").ap()
    O = nc.dram_tensor("O", [2, 2 * R, DOUT], F32, kind="ExternalOutput").ap()
    with tile.TileContext(nc) as tc:
        build_attn(tc, X, WQ, WK, WV, WO, O, N, D, DOUT)
    nc.compile()
    return nc


_NC_CACHE = {}


def _get_nc():
    if "full" not in _NC_CACHE:
        _NC_CACHE["full"] = build_nc()
    return _NC_CACHE["full"]


class _PjrtRunner:
    """Cached jitted SPMD executor (mirrors bass2jax.run_bass_via_pjrt but
    keeps the jitted callable so repeat calls skip re-trace/re-compile)."""

    def __init__(self, nc, n_cores=8):
        import jax
        from jax.experimental.shard_map import shard_map
        from jax.sharding import Mesh, PartitionSpec
        from concourse import bass2jax

        bass2jax.install_neuronx_cc_hook()
        self.n_cores = n_cores
        partition_name = (
            nc.partition_id_tensor.name if nc.partition_id_tensor else None
        )
        in_names, out_names, out_avals, zero_outs = [], [], [], []
        for alloc in nc.m.functions[0].allocations:
            if not isinstance(alloc, mybir.MemoryLocationSet):
                continue
            name = alloc.memorylocations[0].name
            if alloc.kind == "ExternalInput":
                if name != partition_name:
                    in_names.append(name)
            elif alloc.kind == "ExternalOutput":
                out_names.append(name)
                shape = tuple(alloc.tensor_shape)
                dtype = mybir.dt.np(alloc.dtype)
                out_avals.append(jax.core.ShapedArray(shape, dtype))
                zero_outs.append(np.zeros(shape, dtype))
        self.in_names = in_names
        self.out_names = out_names
        self.out_avals = out_avals
        self.zero_outs = zero_outs
        n_params = len(in_names)
        n_outs = len(out_names)
        all_names = list(in_names + out_names)
        if partition_name is not None:
            all_names.append(partition_name)
        all_names = tuple(all_names)

        def _body(*args):
            operands = list(args)
            if partition_name is not None:
                operands.append(bass2jax.partition_id_tensor())
            outs = bass2jax._bass_exec_p.bind(
                *operands,
                out_avals=tuple(out_avals),
                in_names=all_names,
                out_names=tuple(out_names),
                lowering_input_output_aliases=(),
                sim_require_finite=True,
                sim_require_nnan=True,
                nc=nc,
            )
            return tuple(outs)

        devices = jax.devices()[:n_cores]
        mesh = Mesh(np.asarray(devices), ("core",))
        donate = tuple(range(n_params, n_params + n_outs))
        self._fn = jax.jit(
            shard_map(
                _body,
                mesh=mesh,
                in_specs=(PartitionSpec("core"),) * (n_params + n_outs),
                out_specs=(PartitionSpec("core"),) * n_outs,
                check_rep=False,
            ),
            donate_argnums=donate,
            keep_unused=True,
        )

    def __call__(self, in_maps):
        import jax

        n = self.n_cores
        concat_in = [
            np.concatenate([np.asarray(m[nm]) for m in in_maps], axis=0)
            for nm in self.in_names
        ]
        concat_zeros = [
            np.zeros((n * z.shape[0], *z.shape[1:]), z.dtype) for z in self.zero_outs
        ]
        outs = self._fn(*concat_in, *concat_zeros)
        outs = [np.asarray(o) for o in jax.block_until_ready(outs)]
        return [
            {
                nm: outs[i].reshape(n, *self.out_avals[i].shape)[c]
                for i, nm in enumerate(self.out_names)
            }
            for c in range(n)
        ]


def _get_runner():
    if "runner" not in _NC_CACHE:
        _NC_CACHE["runner"] = _PjrtRunner(_get_nc())
    return _NC_CACHE["runner"]


def _make_in_maps(X, W_Q, W_K, W_V, W_O):
    X = np.ascontiguousarray(np.asarray(X), dtype=np.float32)
    W_Q = np.asarray(W_Q, dtype=np.float32)
    W_K = np.asarray(W_K, dtype=np.float32)
    W_V = np.asarray(W_V, dtype=np.float32)
    W_O = np.ascontiguousarray(np.asarray(W_O), dtype=np.float32)
    in_maps = []
    for c in range(8):
        wq = np.ascontiguousarray(
            np.concatenate([W_Q[2 * c], W_Q[2 * c + 1]], axis=1), dtype=np.float32
        )
        wk = np.ascontiguousarray(
            np.concatenate([W_K[2 * c], W_K[2 * c + 1]], axis=1), dtype=np.float32
        )
        wv = np.ascontiguousarray(
            np.concatenate([W_V[2 * c], W_V[2 * c + 1]], axis=1), dtype=np.float32
        )
        in_maps.append({"X": X, "WQ": wq, "WK": wk, "WV": wv, "WO": W_O})
    return in_maps


def kernel_with_results(X, W_Q, W_K, W_V, W_O, **run_kwargs):
    """Run via run_bass_kernel_spmd (supports trace kwargs); returns results."""
    nc = _get_nc()
    in_maps = _make_in_maps(X, W_Q, W_K, W_V, W_O)
    res = run_bass_kernel_spmd(nc, in_maps, core_ids=list(range(8)), **run_kwargs)
    return np.concatenate([r["O"] for r in res.results], axis=1), res


def kernel(X, W_Q, W_K, W_V, W_O):
    """Full-input entry point. X [2,2048,1024], W_Q/K/V [16,1024,64],
    W_O [1024,1024] -> [2,2048,1024] fp32."""
    try:
        runner = _get_runner()
        results = runner(_make_in_maps(X, W_Q, W_K, W_V, W_O))
        return np.concatenate([r["O"] for r in results], axis=1)
    except Exception:
        out, _ = kernel_with_results(X, W_Q, W_K, W_V, W_O)
        return out



# revision 5
# speedup vs baseline: 1.0639x; 1.0639x over previous
"""Trainium2 Bass kernel for nn_CausalSelfAttention_73358041415963.

Math (literal reference semantics):
  Q/K/V = per-head projections of X;  S = Q @ K^T (no scale, no mask)
  A = softmax(S, axis=QUERY)  -> each key-column normalized over queries
  AV = E @ (V / n) with n[key] = sum_q E^T[key, q];  literal reshape
  (B,H,N,DV)->(B,N,H*DV) maps head h to output rows [h*128,(h+1)*128);
  out = AV_r @ W_O.  Head-sharding needs no collectives.

Sharding: 8 cores x 2 heads. Each core gets full X (bf16), its 2 heads'
W_Q/W_K/W_V packed [D,128] bf16, full W_O bf16. Core c returns output
rows [256c, 256c+256) as fp32.

Schedule (per core): ACT-paced exp stream (64 exps of [128,2048]) with
everything else hidden under it:
  - X^T produced by DMA-engine xbar transposes (bf16), no PE transposes
  - scores [key, q] via 4x512-col bf16 matmuls into a 4-bank PSUM tile
  - AV out [q, dv] (full 128-contraction, 64-col bf16 matmuls) into a
    pre-zeroed 2-bank PSUM accumulator
  - W_O contraction 128-deep: AV re-transposed on PE (cheap, bf16) and
    parity-merged into [128,128] lhsT tiles
  - batch-1 projections pumped as background PE work under batch-0 exps
"""

import numpy as np

import concourse.tile as tile
from concourse import bacc, mybir
from concourse.bass_utils import run_bass_kernel_spmd
from concourse.masks import make_identity

F32 = mybir.dt.float32
F32R = mybir.dt.float32r
BF16 = mybir.dt.bfloat16
P = 128
AF = mybir.ActivationFunctionType


def build_attn(tc, X, WQ, WK, WV, WO, O, N, D, DOUT):
    """Emit the per-core kernel into TileContext tc.

    X:  [2, N, D] bf16 (full input)
    WQ/WK/WV: [D, 128] bf16 (2 local heads packed on the last axis)
    WO: [D, DOUT] bf16
    O:  [2, 2*(N//16), DOUT] fp32 output rows for the 2 local heads
    """
    nc = tc.nc
    B, HL = 2, 2
    DCH = D // 128            # contraction chunks over model dim
    NC4 = N // 512            # 512-col chunks of sequence (proj granularity)
    JKB = N // 128            # key blocks
    QB = N // 128             # query blocks
    GW = DCH                  # W_O contraction groups of 128

    with (
        tc.tile_pool(name="pers", bufs=1) as pp,
        tc.tile_pool(name="work", bufs=1) as sp,
        tc.tile_pool(name="ps", bufs=1, space="PSUM") as ps,
    ):
        # ---------------- prologue: constants + weight loads ----------
        ident = pp.tile([P, P], F32, tag="ident", name="ident")
        make_identity(nc, ident)
        identr = pp.tile([P, P], F32R, tag="identr", name="identr")
        nc.vector.tensor_copy(identr, ident)
        zz = pp.tile([1, 512], BF16, tag="zz", name="zz")
        nc.gpsimd.memset(zz, 0.0)
        # Force the ACT Exp table load during the prologue.
        warm = pp.tile([P, 1], F32, tag="warm", name="warm")
        nc.scalar.activation(warm, ident[:, 0:1], AF.Exp)

        wo_sb = pp.tile([P, GW, DOUT], BF16, tag="wo", name="wo_sb")
        nc.sync.dma_start(wo_sb, WO.rearrange("(g p) d -> p g d", p=P))
        wq_sb = pp.tile([P, DCH, P], BF16, tag="wq", name="wq_sb")
        wk_sb = pp.tile([P, DCH, P], BF16, tag="wk", name="wk_sb")
        wv_sb = pp.tile([P, DCH, P], BF16, tag="wv", name="wv_sb")
        nc.sync.dma_start(wq_sb, WQ.rearrange("(dc p) m -> p dc m", p=P))
        nc.sync.dma_start(wk_sb, WK.rearrange("(dc p) m -> p dc m", p=P))
        nc.sync.dma_start(wv_sb, WV.rearrange("(dc p) m -> p dc m", p=P))

        # persistent qT/kT (bf16 [dk-packed, n]) and V natural [key, dv]
        qT, kT, v_sb = [], [], []
        for b in range(B):
            qT.append(pp.tile([P, N], BF16, tag=f"qT{b}", name=f"qT{b}"))
            kT.append(pp.tile([P, N], BF16, tag=f"kT{b}", name=f"kT{b}"))
            v_sb.append(
                pp.tile([P, JKB, P], BF16, tag=f"v{b}", name=f"v{b}")
            )

        # ------------- X^T via DMA xbar transposes (bf16) -------------
        # xc[(b, dc, ci)] = X^T chunk [128 d, 512 n]
        xc = {}

        def emit_xt(b, dc, ci):
            t = sp.tile([P, 512], BF16, tag="xc", bufs=40, name="xc")
            nc.sync.dma_start_transpose(
                t, X[b, ci * 512 : (ci + 1) * 512, dc * 128 : (dc + 1) * 128]
            )
            xc[(b, dc, ci)] = t

        for b in range(B):
            for ci in range(NC4):
                for dc in range(DCH):
                    emit_xt(b, dc, ci)

        # ------------- projection groups (PE + DVE drain) -------------
        vtc = {}  # staged V^T chunks awaiting DMA transpose

        def emit_proj_group(b, which, ci):
            w_sb = {"q": wq_sb, "k": wk_sb, "v": wv_sb}[which]
            pj = ps.tile([P, 512], F32, tag="pj", bufs=2, name="pj")
            for dc in range(DCH):
                nc.tensor.matmul(
                    pj, w_sb[:, dc, :], xc[(b, dc, ci)],
                    start=(dc == 0), stop=(dc == DCH - 1),
                )
            if which == "q":
                nc.vector.tensor_copy(qT[b][:, ci * 512 : (ci + 1) * 512], pj)
            elif which == "k":
                nc.vector.tensor_copy(kT[b][:, ci * 512 : (ci + 1) * 512], pj)
            else:
                vt = sp.tile([P, 512], BF16, tag="vtc", bufs=2, name="vtc")
                nc.vector.tensor_copy(vt, pj)
                nc.sync.dma_start_transpose(
                    v_sb[b][:, ci * 4 : (ci + 1) * 4, :], vt
                )

        # background PE work queue (proj groups pumped under the exp
        # stream); scores ordering requirements are honored by emission
        # order + Tile-inserted semaphores.
        bgq = []
        for which, ci in [("v", 0), ("k", 1), ("v", 1), ("k", 2), ("v", 2),
                          ("k", 3), ("v", 3)]:
            bgq.append((0, which, ci))
        for ci in range(NC4):
            for which in ("q", "k", "v"):
                bgq.append((1, which, ci))

        def pump_bg(k=1):
            for _ in range(k):
                if bgq:
                    b_, w_, c_ = bgq.pop(0)
                    emit_proj_group(b_, w_, c_)

        # head: enough of qT/kT(b0) for the first scores
        for which, ci in [("q", 0), ("k", 0), ("q", 1), ("q", 2), ("q", 3)]:
            emit_proj_group(0, which, ci)

        # ---------------- phase A: ACT-paced slot pipeline -------------
        def emit_av_pass(prev, avq, jk):
            """One jk accumulation pass of the previous slot's AV."""
            es, vss = prev["es"], prev["vss"]
            for qb in range(QB):
                nc.tensor.matmul(
                    avq[:, qb * 64 : qb * 64 + 64],
                    es[jk][:, qb * 128 : (qb + 1) * 128],
                    vss[jk],
                    start=False, stop=(jk == JKB - 1),
                    skip_group_check=True,
                )

        def emit_wo(prev, avq):
            """Drain AV accumulator, transpose, parity-merge, W_O, out."""
            b, h = prev["b"], prev["h"]
            avs = sp.tile([P, QB, 64], F32R, tag="avs", bufs=2, name="avs")
            nc.vector.tensor_copy(avs, avq)
            avT = sp.tile([64, QB, P], BF16, tag="avT", bufs=2, name="avT")
            for grp in range(4):
                tp = ps.tile([P, 512], F32R, tag="pj", bufs=2, name="tp")
                for k in range(4):
                    nc.tensor.transpose(
                        tp[0:64, k * 128 : (k + 1) * 128],
                        avs[:, grp * 4 + k, :],
                        identr,
                    )
                nc.vector.tensor_copy(
                    avT[:, grp * 4 : (grp + 1) * 4, :], tp[0:64, :]
                )
            lts = []
            for g in range(GW):
                lt = sp.tile([P, P], BF16, tag="lt", bufs=10, name="lt")
                nc.vector.tensor_copy(lt[0:64, :], avT[:, :, 2 * g :: 16])
                nc.vector.tensor_copy(lt[64:128, :], avT[:, :, 2 * g + 1 :: 16])
                lts.append(lt)
            for dch in range(DOUT // 512):
                op = ps.tile([P, 512], F32, tag="pj", bufs=2, name="op")
                for g in range(GW):
                    nc.tensor.matmul(
                        op, lts[g], wo_sb[:, g, dch * 512 : (dch + 1) * 512],
                        start=(g == 0), stop=(g == GW - 1),
                    )
                ot = sp.tile([P, 512], F32, tag="ot", bufs=3, name="ot")
                nc.vector.tensor_copy(ot, op)
                nc.sync.dma_start(
                    O[b, h * P : (h + 1) * P, dch * 512 : (dch + 1) * 512], ot
                )

        prev = None
        for b in range(B):
            for h in range(HL):
                hs = slice(h * 64, (h + 1) * 64)
                avq_prev = None
                if prev is not None:
                    avq_prev = ps.tile(
                        [P, QB * 64], F32, tag="avq", bufs=1, name="avq"
                    )
                    # zero via PE (start=True full-bank writes)
                    for zc in range(2):
                        nc.tensor.matmul(
                            avq_prev[:, zc * 512 : (zc + 1) * 512],
                            zz[:, 0:128], zz[:, :],
                            start=True, stop=True, skip_group_check=True,
                        )
                es, nss = [], []
                for jk in range(JKB):
                    st = ps.tile([P, N], F32, tag="st", bufs=1, name="st")
                    for qc in range(4):
                        nc.tensor.matmul(
                            st[:, qc * 512 : (qc + 1) * 512],
                            kT[b][hs, jk * 128 : (jk + 1) * 128],
                            qT[b][hs, qc * 512 : (qc + 1) * 512],
                            start=True, stop=True,
                        )
                    e_t = sp.tile([P, N], BF16, tag="e", bufs=18, name="e")
                    ns = sp.tile([P, 1], F32, tag="ns", bufs=20, name="ns")
                    nc.scalar.activation(e_t, st, AF.Exp, accum_out=ns)
                    es.append(e_t)
                    nss.append(ns)
                    if prev is not None:
                        emit_av_pass(prev, avq_prev, jk)
                    pump_bg(1)
                # normalizers + scaled V rows for this slot (DVE, end of
                # slot so proj drains are never blocked behind them)
                vss = []
                for jk in range(JKB):
                    nr = sp.tile([P, 1], F32, tag="nr", bufs=20, name="nr")
                    nc.vector.reciprocal(nr, nss[jk])
                    vs = sp.tile([P, 64], BF16, tag="vs", bufs=20, name="vs")
                    nc.vector.tensor_scalar_mul(vs, v_sb[b][:, jk, hs], nr)
                    vss.append(vs)
                if prev is not None:
                    emit_wo(prev, avq_prev)
                prev = {"b": b, "h": h, "es": es, "vss": vss}

        # epilogue: AV + W_O for the final slot
        pump_bg(len(bgq))
        avq_prev = ps.tile([P, QB * 64], F32, tag="avq", bufs=1, name="avq")
        for zc in range(2):
            nc.tensor.matmul(
                avq_prev[:, zc * 512 : (zc + 1) * 512],
                zz[:, 0:128], zz[:, :],
                start=True, stop=True, skip_group_check=True,
            )
        for jk in range(JKB):
            emit_av_pass(prev, avq_prev, jk)
        emit_wo(prev, avq_prev)


def build_nc(N=2048, D=1024, DOUT=1024, enable_asserts=False):
    """Build and compile the per-core Bass module. Returns nc."""
    nc = bacc.Bacc(
        "TRN2",
        target_bir_lowering=False,
        debug=False,
        enable_asserts=enable_asserts,
    )
    R = N // 16
    X = nc.dram_tensor("X", [2, N, D], BF16, kind="ExternalInput").ap()
    WQ = nc.dram_tensor("WQ", [D, 128], BF16, kind="ExternalInput").ap()
    WK = nc.dram_tensor("WK", [D, 128], BF16, kind="ExternalInput").ap()
    WV = nc.dram_tensor("WV", [D, 128], BF16, kind="ExternalInput").ap()
    WO = nc.dram_tensor("WO", [D, DOUT], BF16, kind="ExternalInput").ap()
    O = nc.dram_tensor("O", [2, 2 * R, DOUT], F32, kind="ExternalOutput").ap()
    with tile.TileContext(nc) as tc:
        build_attn(tc, X, WQ, WK, WV, WO, O, N, D, DOUT)
    nc.compile()
    return nc


_NC_CACHE = {}


def _get_nc():
    if "full" not in _NC_CACHE:
        _NC_CACHE["full"] = build_nc()
    return _NC_CACHE["full"]


class _PjrtRunner:
    """Cached jitted SPMD executor (mirrors bass2jax.run_bass_via_pjrt but
    keeps the jitted callable so repeat calls skip re-trace/re-compile)."""

    def __init__(self, nc, n_cores=8):
        import jax
        from jax.experimental.shard_map import shard_map
        from jax.sharding import Mesh, PartitionSpec
        from concourse import bass2jax

        bass2jax.install_neuronx_cc_hook()
        self.n_cores = n_cores
        partition_name = (
            nc.partition_id_tensor.name if nc.partition_id_tensor else None
        )
        in_names, out_names, out_avals, zero_outs = [], [], [], []
        for alloc in nc.m.functions[0].allocations:
            if not isinstance(alloc, mybir.MemoryLocationSet):
                continue
            name = alloc.memorylocations[0].name
            if alloc.kind == "ExternalInput":
                if name != partition_name:
                    in_names.append(name)
            elif alloc.kind == "ExternalOutput":
                out_names.append(name)
                shape = tuple(alloc.tensor_shape)
                dtype = mybir.dt.np(alloc.dtype)
                out_avals.append(jax.core.ShapedArray(shape, dtype))
                zero_outs.append(np.zeros(shape, dtype))
        self.in_names = in_names
        self.out_names = out_names
        self.out_avals = out_avals
        self.zero_outs = zero_outs
        n_params = len(in_names)
        n_outs = len(out_names)
        all_names = list(in_names + out_names)
        if partition_name is not None:
            all_names.append(partition_name)
        all_names = tuple(all_names)

        def _body(*args):
            operands = list(args)
            if partition_name is not None:
                operands.append(bass2jax.partition_id_tensor())
            outs = bass2jax._bass_exec_p.bind(
                *operands,
                out_avals=tuple(out_avals),
                in_names=all_names,
                out_names=tuple(out_names),
                lowering_input_output_aliases=(),
                sim_require_finite=True,
                sim_require_nnan=True,
                nc=nc,
            )
            return tuple(outs)

        devices = jax.devices()[:n_cores]
        mesh = Mesh(np.asarray(devices), ("core",))
        donate = tuple(range(n_params, n_params + n_outs))
        self._fn = jax.jit(
            shard_map(
                _body,
                mesh=mesh,
                in_specs=(PartitionSpec("core"),) * (n_params + n_outs),
                out_specs=(PartitionSpec("core"),) * n_outs,
                check_rep=False,
            ),
            donate_argnums=donate,
            keep_unused=True,
        )

    def __call__(self, in_maps):
        import jax

        n = self.n_cores
        concat_in = [
            np.concatenate([np.asarray(m[nm]) for m in in_maps], axis=0)
            for nm in self.in_names
        ]
        concat_zeros = [
            np.zeros((n * z.shape[0], *z.shape[1:]), z.dtype) for z in self.zero_outs
        ]
        outs = self._fn(*concat_in, *concat_zeros)
        outs = [np.asarray(o) for o in jax.block_until_ready(outs)]
        return [
            {
                nm: outs[i].reshape(n, *self.out_avals[i].shape)[c]
                for i, nm in enumerate(self.out_names)
            }
            for c in range(n)
        ]


def _get_runner():
    if "runner" not in _NC_CACHE:
        _NC_CACHE["runner"] = _PjrtRunner(_get_nc())
    return _NC_CACHE["runner"]


def _make_in_maps(X, W_Q, W_K, W_V, W_O):
    import ml_dtypes

    bf = ml_dtypes.bfloat16
    Xb = np.ascontiguousarray(np.asarray(X, dtype=np.float32)).astype(bf)
    W_Q = np.asarray(W_Q, dtype=np.float32)
    W_K = np.asarray(W_K, dtype=np.float32)
    W_V = np.asarray(W_V, dtype=np.float32)
    WOb = np.ascontiguousarray(np.asarray(W_O, dtype=np.float32)).astype(bf)
    in_maps = []
    for c in range(8):
        wq = np.ascontiguousarray(
            np.concatenate([W_Q[2 * c], W_Q[2 * c + 1]], axis=1)
        ).astype(bf)
        wk = np.ascontiguousarray(
            np.concatenate([W_K[2 * c], W_K[2 * c + 1]], axis=1)
        ).astype(bf)
        wv = np.ascontiguousarray(
            np.concatenate([W_V[2 * c], W_V[2 * c + 1]], axis=1)
        ).astype(bf)
        in_maps.append({"X": Xb, "WQ": wq, "WK": wk, "WV": wv, "WO": WOb})
    return in_maps


def kernel_with_results(X, W_Q, W_K, W_V, W_O, **run_kwargs):
    """Run via run_bass_kernel_spmd (supports trace kwargs); returns results."""
    nc = _get_nc()
    in_maps = _make_in_maps(X, W_Q, W_K, W_V, W_O)
    res = run_bass_kernel_spmd(nc, in_maps, core_ids=list(range(8)), **run_kwargs)
    return np.concatenate([r["O"] for r in res.results], axis=1), res


def kernel(X, W_Q, W_K, W_V, W_O):
    """Full-input entry point. X [2,2048,1024], W_Q/K/V [16,1024,64],
    W_O [1024,1024] -> [2,2048,1024] fp32."""
    try:
        runner = _get_runner()
        results = runner(_make_in_maps(X, W_Q, W_K, W_V, W_O))
        return np.concatenate([r["O"] for r in results], axis=1)
    except Exception:
        out, _ = kernel_with_results(X, W_Q, W_K, W_V, W_O)
        return out


# revision 15
# speedup vs baseline: 1.4727x; 1.3842x over previous
"""Trainium2 Bass kernel for nn_CausalSelfAttention_73358041415963.

Math (literal reference semantics):
  Q/K/V = per-head projections of X;  S = Q @ K^T (no scale, no mask)
  A = softmax(S, axis=QUERY)  -> each key-column normalized over queries
  AV = E @ (V / n) with n[key] = sum_q E^T[key, q];  literal reshape
  (B,H,N,DV)->(B,N,H*DV) maps head h to output rows [h*128,(h+1)*128);
  out = AV_r @ W_O.  Head-sharding needs no collectives.

Sharding: 8 cores x 2 heads. Each core gets full X (bf16), its 2 heads'
W_Q/W_K/W_V packed [D,128] bf16, full W_O bf16. Core c returns output
rows [256c, 256c+256) as fp32.

Schedule (per core): ACT-paced exp stream (64 exps of [128,2048]) with
everything else hidden under it:
  - X^T produced by DMA-engine xbar transposes (bf16), no PE transposes
  - scores [key, q] via 4x512-col bf16 matmuls into a 4-bank PSUM tile
  - AV out [q, dv] (full 128-contraction, 64-col bf16 matmuls) into a
    pre-zeroed 2-bank PSUM accumulator
  - W_O contraction 128-deep: AV re-transposed on PE (cheap, bf16) and
    parity-merged into [128,128] lhsT tiles
  - batch-1 projections pumped as background PE work under batch-0 exps
"""

import numpy as np

import concourse.tile as tile
from concourse import bacc, mybir
from concourse.bass_utils import run_bass_kernel_spmd
from concourse.masks import make_identity

F32 = mybir.dt.float32
F32R = mybir.dt.float32r
BF16 = mybir.dt.bfloat16
P = 128
AF = mybir.ActivationFunctionType


def build_attn(tc, X, WQ, WK, WV, WO, O, N, D, DOUT):
    """Emit the per-core kernel into TileContext tc.

    X:  [2, N, D] bf16 (full input)
    WQ/WK/WV: [D, 128] bf16 (2 local heads packed on the last axis)
    WO: [D, DOUT] bf16
    O:  [2, 2*(N//16), DOUT] fp32 output rows for the 2 local heads
    """
    nc = tc.nc
    B, HL = 2, 2
    DCH = D // 128            # contraction chunks over model dim
    NC4 = N // 512            # 512-col chunks of sequence (proj granularity)
    JKB = N // 128            # key blocks
    QB = N // 128             # query blocks
    GW = DCH                  # W_O contraction groups of 128

    with (
        tc.tile_pool(name="pers", bufs=1) as pp,
        tc.tile_pool(name="work", bufs=1) as sp,
        tc.tile_pool(name="ps", bufs=1, space="PSUM") as ps,
    ):
        # ---------------- prologue: constants + weight loads ----------
        ident = pp.tile([P, P], F32, tag="ident", name="ident")
        make_identity(nc, ident)
        identr = pp.tile([P, P], F32R, tag="identr", name="identr")
        nc.vector.tensor_copy(identr, ident)
        zz = pp.tile([1, 512], BF16, tag="zz", name="zz")
        nc.gpsimd.memset(zz, 0.0)
        # Force the ACT Exp table load during the prologue.
        warm = pp.tile([P, 1], F32, tag="warm", name="warm")
        nc.scalar.activation(warm, ident[:, 0:1], AF.Exp)

        wq_sb = pp.tile([P, DCH, P], BF16, tag="wq", name="wq_sb")
        wk_sb = pp.tile([P, DCH, P], BF16, tag="wk", name="wk_sb")
        wv_sb = pp.tile([P, DCH, P], BF16, tag="wv", name="wv_sb")
        nc.sync.dma_start(wq_sb, WQ.rearrange("(dc p) m -> p dc m", p=P))
        nc.sync.dma_start(wk_sb, WK.rearrange("(dc p) m -> p dc m", p=P))
        nc.sync.dma_start(wv_sb, WV.rearrange("(dc p) m -> p dc m", p=P))

        # persistent qT/kT (bf16 [dk-packed, n]) and V natural [key, dv]
        qT, kT, v_sb = [], [], []
        for b in range(B):
            qT.append(pp.tile([P, N], BF16, tag=f"qT{b}", name=f"qT{b}"))
            kT.append(pp.tile([P, N], BF16, tag=f"kT{b}", name=f"kT{b}"))
            v_sb.append(
                pp.tile([P, JKB, P], BF16, tag=f"v{b}", name=f"v{b}")
            )

        # ------------- X^T via DMA xbar transposes (bf16) -------------
        # xh[(b, dc, half)] = X^T half-row chunk [128 d, 1024 n]
        xh = {}

        def emit_xt(b, dc, half):
            t = sp.tile([P, 1024], BF16, tag="xc", bufs=24, name="xc")
            nc.sync.dma_start_transpose(
                t, X[b, half * 1024 : (half + 1) * 1024,
                     dc * 128 : (dc + 1) * 128]
            )
            xh[(b, dc, half)] = t

        for b in range(B):
            for half in range(2):
                for dc in range(DCH):
                    emit_xt(b, dc, half)

        # W_O load after the X^T transposes so it doesn't delay the head
        wo_sb = pp.tile([P, GW, DOUT], BF16, tag="wo", name="wo_sb")
        nc.sync.dma_start(wo_sb, WO.rearrange("(g p) d -> p g d", p=P))

        # ------------- projection groups (PE + DVE drain) -------------
        vtc = {}  # staged V^T chunks awaiting DMA transpose

        def emit_proj_group(b, which, ci):
            w_sb = {"q": wq_sb, "k": wk_sb, "v": wv_sb}[which]
            pj = ps.tile([P, 512], F32, tag="pj", bufs=2, name="pj")
            half, off = ci // 2, (ci % 2) * 512
            for dc in range(DCH):
                nc.tensor.matmul(
                    pj, w_sb[:, dc, :], xh[(b, dc, half)][:, off : off + 512],
                    start=(dc == 0), stop=(dc == DCH - 1),
                )
            if which == "q":
                nc.vector.tensor_copy(qT[b][:, ci * 512 : (ci + 1) * 512], pj)
            elif which == "k":
                nc.vector.tensor_copy(kT[b][:, ci * 512 : (ci + 1) * 512], pj)
            else:
                vt = sp.tile([P, 512], BF16, tag="vtc", bufs=2, name="vtc")
                nc.vector.tensor_copy(vt, pj)
                nc.sync.dma_start_transpose(
                    v_sb[b][:, ci * 4 : (ci + 1) * 4, :], vt
                )

        # background PE work queue (proj groups pumped under the exp
        # stream); scores ordering requirements are honored by emission
        # order + Tile-inserted semaphores.
        bgq = []
        for which, ci in [("v", 0), ("k", 1), ("v", 1), ("k", 2), ("v", 2),
                          ("k", 3), ("v", 3)]:
            bgq.append((0, which, ci))
        # batch-1 groups ordered so early pumps only touch xh halves that
        # have already ring-cycled (half 0 first)
        for ci in range(NC4):
            for which in ("q", "k", "v"):
                bgq.append((1, which, ci))

        def pump_bg(k=1):
            for _ in range(k):
                if bgq:
                    b_, w_, c_ = bgq.pop(0)
                    emit_proj_group(b_, w_, c_)

        # head: enough of qT/kT(b0) for the first scores
        for which, ci in [("q", 0), ("k", 0), ("q", 1), ("q", 2), ("q", 3)]:
            emit_proj_group(0, which, ci)

        # ---------------- phase A: ACT-paced slot pipeline -------------
        def emit_av_pass(prev, avq, jk):
            """One jk accumulation pass of the previous slot's AV."""
            es, vss = prev["es"], prev["vss"]
            for qb in range(QB):
                nc.tensor.matmul(
                    avq[:, qb * 64 : qb * 64 + 64],
                    es[jk][:, qb * 128 : (qb + 1) * 128],
                    vss[jk],
                    start=False, stop=(jk == JKB - 1),
                    skip_group_check=True,
                )

        def emit_wo(prev, avq):
            """Drain AV accumulator, transpose, parity-merge, W_O, out."""
            b, h = prev["b"], prev["h"]
            avs = sp.tile([P, QB, 64], F32R, tag="avs", bufs=2, name="avs")
            nc.vector.tensor_copy(avs, avq)
            avT = sp.tile([64, QB, P], BF16, tag="avT", bufs=2, name="avT")
            for grp in range(4):
                tp = ps.tile([P, 512], F32R, tag="pj", bufs=2, name="tp")
                for k in range(4):
                    nc.tensor.transpose(
                        tp[0:64, k * 128 : (k + 1) * 128],
                        avs[:, grp * 4 + k, :],
                        identr,
                    )
                nc.vector.tensor_copy(
                    avT[:, grp * 4 : (grp + 1) * 4, :], tp[0:64, :]
                )
            lts = []
            for g in range(GW):
                lt = sp.tile([P, P], BF16, tag="lt", bufs=10, name="lt")
                nc.vector.tensor_copy(lt[0:64, :], avT[:, :, 2 * g :: 16])
                nc.vector.tensor_copy(lt[64:128, :], avT[:, :, 2 * g + 1 :: 16])
                lts.append(lt)
            for dch in range(DOUT // 512):
                op = ps.tile([P, 512], F32, tag="pj", bufs=2, name="op")
                for g in range(GW):
                    nc.tensor.matmul(
                        op, lts[g], wo_sb[:, g, dch * 512 : (dch + 1) * 512],
                        start=(g == 0), stop=(g == GW - 1),
                    )
                ot = sp.tile([P, 512], F32, tag="ot", bufs=3, name="ot")
                nc.vector.tensor_copy(ot, op)
                nc.sync.dma_start(
                    O[b, h * P : (h + 1) * P, dch * 512 : (dch + 1) * 512], ot
                )

        prev = None
        for b in range(B):
            for h in range(HL):
                hs = slice(h * 64, (h + 1) * 64)
                avq_prev = None
                if prev is not None:
                    avq_prev = ps.tile(
                        [P, QB * 64], F32, tag="avq", bufs=1, name="avq"
                    )
                    # zero via PE (start=True full-bank writes)
                    for zc in range(2):
                        nc.tensor.matmul(
                            avq_prev[:, zc * 512 : (zc + 1) * 512],
                            zz[:, 0:128], zz[:, :],
                            start=True, stop=True, skip_group_check=True,
                        )
                es = []
                for jk in range(JKB):
                    # ping-pong 2-bank score tiles: scores(jk+1) into stA
                    # overlap exp_b(jk) reading stB -> gapless ACT stream
                    sta = ps.tile([P, N // 2], F32, tag="stA", bufs=1,
                                  name="sta")
                    stb = ps.tile([P, N // 2], F32, tag="stB", bufs=1,
                                  name="stb")
                    for qc in range(2):
                        nc.tensor.matmul(
                            sta[:, qc * 512 : (qc + 1) * 512],
                            kT[b][hs, jk * 128 : (jk + 1) * 128],
                            qT[b][hs, qc * 512 : (qc + 1) * 512],
                            start=True, stop=True,
                        )
                    for qc in range(2):
                        nc.tensor.matmul(
                            stb[:, qc * 512 : (qc + 1) * 512],
                            kT[b][hs, jk * 128 : (jk + 1) * 128],
                            qT[b][hs, 1024 + qc * 512 : 1024 + (qc + 1) * 512],
                            start=True, stop=True,
                        )
                    e_t = sp.tile([P, N], BF16, tag="e", bufs=18, name="e")
                    nc.scalar.activation(e_t[:, 0 : N // 2], sta, AF.Exp)
                    nc.scalar.activation(e_t[:, N // 2 : N], stb, AF.Exp)
                    es.append(e_t)
                    if prev is not None:
                        emit_av_pass(prev, avq_prev, jk)
                    pump_bg(1)
                # normalizers + scaled V rows for this slot: row-sum of E
                # via a fast-mode DVE tensor_scalar with accum_out (x1.0
                # into a dummy, sum lands in ns), then 1/n and V scaling
                vss = []
                for jk in range(JKB):
                    ns = sp.tile([P, 1], F32, tag="ns", bufs=20, name="ns")
                    dum = sp.tile([P, N], BF16, tag="dum", bufs=1, name="dum")
                    nc.vector.tensor_scalar(
                        dum, es[jk], 1.0, 0.0, mybir.AluOpType.mult,
                        op1=mybir.AluOpType.add, accum_out=ns,
                    )
                    nr = sp.tile([P, 1], F32, tag="nr", bufs=20, name="nr")
                    nc.vector.reciprocal(nr, ns)
                    vs = sp.tile([P, 64], BF16, tag="vs", bufs=20, name="vs")
                    nc.vector.tensor_scalar_mul(vs, v_sb[b][:, jk, hs], nr)
                    vss.append(vs)
                if prev is not None:
                    emit_wo(prev, avq_prev)
                prev = {"b": b, "h": h, "es": es, "vss": vss}

        # epilogue: AV + W_O for the final slot
        pump_bg(len(bgq))
        avq_prev = ps.tile([P, QB * 64], F32, tag="avq", bufs=1, name="avq")
        for zc in range(2):
            nc.tensor.matmul(
                avq_prev[:, zc * 512 : (zc + 1) * 512],
                zz[:, 0:128], zz[:, :],
                start=True, stop=True, skip_group_check=True,
            )
        for jk in range(JKB):
            emit_av_pass(prev, avq_prev, jk)
        emit_wo(prev, avq_prev)


def build_nc(N=2048, D=1024, DOUT=1024, enable_asserts=False):
    """Build and compile the per-core Bass module. Returns nc."""
    nc = bacc.Bacc(
        "TRN2",
        target_bir_lowering=False,
        debug=False,
        enable_asserts=enable_asserts,
    )
    R = N // 16
    X = nc.dram_tensor("X", [2, N, D], BF16, kind="ExternalInput").ap()
    WQ = nc.dram_tensor("WQ", [D, 128], BF16, kind="ExternalInput").ap()
    WK = nc.dram_tensor("WK", [D, 128], BF16, kind="ExternalInput").ap()
    WV = nc.dram_tensor("WV", [D, 128], BF16, kind="ExternalInput").ap()
    WO = nc.dram_tensor("WO", [D, DOUT], BF16, kind="ExternalInput").ap()
    O = nc.dram_tensor("O", [2, 2 * R, DOUT], F32, kind="ExternalOutput").ap()
    with tile.TileContext(nc) as tc:
        build_attn(tc, X, WQ, WK, WV, WO, O, N, D, DOUT)
    nc.compile()
    return nc


_NC_CACHE = {}


def _get_nc():
    if "full" not in _NC_CACHE:
        _NC_CACHE["full"] = build_nc()
    return _NC_CACHE["full"]


class _PjrtRunner:
    """Cached jitted SPMD executor (mirrors bass2jax.run_bass_via_pjrt but
    keeps the jitted callable so repeat calls skip re-trace/re-compile)."""

    def __init__(self, nc, n_cores=8):
        import jax
        from jax.experimental.shard_map import shard_map
        from jax.sharding import Mesh, PartitionSpec
        from concourse import bass2jax

        bass2jax.install_neuronx_cc_hook()
        self.n_cores = n_cores
        partition_name = (
            nc.partition_id_tensor.name if nc.partition_id_tensor else None
        )
        in_names, out_names, out_avals, zero_outs = [], [], [], []
        for alloc in nc.m.functions[0].allocations:
            if not isinstance(alloc, mybir.MemoryLocationSet):
                continue
            name = alloc.memorylocations[0].name
            if alloc.kind == "ExternalInput":
                if name != partition_name:
                    in_names.append(name)
            elif alloc.kind == "ExternalOutput":
                out_names.append(name)
                shape = tuple(alloc.tensor_shape)
                dtype = mybir.dt.np(alloc.dtype)
                out_avals.append(jax.core.ShapedArray(shape, dtype))
                zero_outs.append(np.zeros(shape, dtype))
        self.in_names = in_names
        self.out_names = out_names
        self.out_avals = out_avals
        self.zero_outs = zero_outs
        n_params = len(in_names)
        n_outs = len(out_names)
        all_names = list(in_names + out_names)
        if partition_name is not None:
            all_names.append(partition_name)
        all_names = tuple(all_names)

        def _body(*args):
            operands = list(args)
            if partition_name is not None:
                operands.append(bass2jax.partition_id_tensor())
            outs = bass2jax._bass_exec_p.bind(
                *operands,
                out_avals=tuple(out_avals),
                in_names=all_names,
                out_names=tuple(out_names),
                lowering_input_output_aliases=(),
                sim_require_finite=True,
                sim_require_nnan=True,
                nc=nc,
            )
            return tuple(outs)

        devices = jax.devices()[:n_cores]
        mesh = Mesh(np.asarray(devices), ("core",))
        donate = tuple(range(n_params, n_params + n_outs))
        self._fn = jax.jit(
            shard_map(
                _body,
                mesh=mesh,
                in_specs=(PartitionSpec("core"),) * (n_params + n_outs),
                out_specs=(PartitionSpec("core"),) * n_outs,
                check_rep=False,
            ),
            donate_argnums=donate,
            keep_unused=True,
        )

    def __call__(self, in_maps):
        import jax

        n = self.n_cores
        concat_in = [
            np.concatenate([np.asarray(m[nm]) for m in in_maps], axis=0)
            for nm in self.in_names
        ]
        concat_zeros = [
            np.zeros((n * z.shape[0], *z.shape[1:]), z.dtype) for z in self.zero_outs
        ]
        outs = self._fn(*concat_in, *concat_zeros)
        outs = [np.asarray(o) for o in jax.block_until_ready(outs)]
        return [
            {
                nm: outs[i].reshape(n, *self.out_avals[i].shape)[c]
                for i, nm in enumerate(self.out_names)
            }
            for c in range(n)
        ]


def _get_runner():
    if "runner" not in _NC_CACHE:
        _NC_CACHE["runner"] = _PjrtRunner(_get_nc())
    return _NC_CACHE["runner"]


def _make_in_maps(X, W_Q, W_K, W_V, W_O):
    import ml_dtypes

    bf = ml_dtypes.bfloat16
    Xb = np.ascontiguousarray(np.asarray(X, dtype=np.float32)).astype(bf)
    W_Q = np.asarray(W_Q, dtype=np.float32)
    W_K = np.asarray(W_K, dtype=np.float32)
    W_V = np.asarray(W_V, dtype=np.float32)
    WOb = np.ascontiguousarray(np.asarray(W_O, dtype=np.float32)).astype(bf)
    in_maps = []
    for c in range(8):
        wq = np.ascontiguousarray(
            np.concatenate([W_Q[2 * c], W_Q[2 * c + 1]], axis=1)
        ).astype(bf)
        wk = np.ascontiguousarray(
            np.concatenate([W_K[2 * c], W_K[2 * c + 1]], axis=1)
        ).astype(bf)
        wv = np.ascontiguousarray(
            np.concatenate([W_V[2 * c], W_V[2 * c + 1]], axis=1)
        ).astype(bf)
        in_maps.append({"X": Xb, "WQ": wq, "WK": wk, "WV": wv, "WO": WOb})
    return in_maps


def kernel_with_results(X, W_Q, W_K, W_V, W_O, **run_kwargs):
    """Run via run_bass_kernel_spmd (supports trace kwargs); returns results."""
    nc = _get_nc()
    in_maps = _make_in_maps(X, W_Q, W_K, W_V, W_O)
    res = run_bass_kernel_spmd(nc, in_maps, core_ids=list(range(8)), **run_kwargs)
    return np.concatenate([r["O"] for r in res.results], axis=1), res


def kernel(X, W_Q, W_K, W_V, W_O):
    """Full-input entry point. X [2,2048,1024], W_Q/K/V [16,1024,64],
    W_O [1024,1024] -> [2,2048,1024] fp32."""
    try:
        runner = _get_runner()
        results = runner(_make_in_maps(X, W_Q, W_K, W_V, W_O))
        return np.concatenate([r["O"] for r in results], axis=1)
    except Exception:
        out, _ = kernel_with_results(X, W_Q, W_K, W_V, W_O)
        return out
